# revision 9
# baseline (speedup 1.0000x reference)
"""Trainium2 Bass kernel for DepthwiseXCorr (SiamRPN-style) model.

Pipeline (per sample): conv3x3+BN+ReLU on kernel & search branches,
depthwise cross-correlation, 1x1 conv + BN + ReLU head, 1x1 conv + bias.

Sharding: data-parallel over batch across 8 NeuronCores (8 samples each),
weights replicated.  BN is folded into conv weights on the host.

Layout on device: channels on SBUF partitions (2 chunks of 128), spatial x
batch on the free dimension.  The search conv runs as 9 shifted bf16
matmuls over 2-sample groups (halves the instruction count).  The
depthwise xcorr is distributed per (sample, channel-chunk) pair across
engines: 't' = TensorE diag(k_tap) matmuls accumulated in PSUM (the 25
diagonals of a pair are built by ONE VectorE tensor_tensor instruction
with broadcast access patterns, 2x 16-bit mode); 'v' = VectorE
scalar_tensor_tensor chains over contiguous 746-elem spans (first tap on
ScalarE); 'm' = ScalarE produces the 25 scaled tap products and VectorE
folds them with 2x fp16 tensor_tensor adds; 's' = split TensorE/VectorE.
Feat/head tensors are fp16 (validated 3.2e-3 end-to-end).  The engine
pipeline is staggered: conv(group g) || xcorr(group g-1) || heads(g-2).
"""

import sys

if "/opt/trn_rl_repo" not in sys.path:
    sys.path.insert(0, "/opt/trn_rl_repo")

from contextlib import ExitStack

import ml_dtypes
import numpy as np

import concourse.bass as bass
import concourse.tile as tile
from concourse import bacc, mybir
from concourse.bass_utils import run_bass_kernel_spmd

EPS = 1e-5
NCORES = 8
B, C, HID, OUT = 64, 256, 256, 10
BPC = B // NCORES  # samples per core
P = 128
KC = C // P  # channel chunks (2)
F32 = mybir.dt.float32
F16 = mybir.dt.float16
BF16 = mybir.dt.bfloat16
AF = mybir.ActivationFunctionType
OP = mybir.AluOpType

# xcorr engine per (b, cc) pair, index p = b*2 + cc
# 't' TensorE diag-matmul | 'v' VectorE STT chain | 'm' ScalarE products +
# VectorE 2x adds | 's' split (SPLIT_TAPS on TensorE, rest on VectorE)
XC_ENGINE = ["s", "m", "m", "v", "v", "m", "m", "v",
             "t", "m", "v", "t", "t", "t", "t", "t"]
SPLIT_TAPS = 15

LAST_RESULTS = None  # BassKernelResults of the most recent run (for profiling)

_prog_cache = {}


def _bcast_ap(base, dims):
    """Raw AP with the tile's partition dim plus explicit [stride, count] dims."""
    return bass.AP(base.tensor, base.offset, [base.ap[0]] + dims)


def _emit(nc, tc, ctx, d):
    """Emit the per-core program.  d maps dram tensor name -> handle."""
    wp = ctx.enter_context(tc.tile_pool(name="weights", bufs=1))
    srp = ctx.enter_context(tc.tile_pool(name="srelu", bufs=1))
    krp = ctx.enter_context(tc.tile_pool(name="krelu", bufs=1))
    kp = ctx.enter_context(tc.tile_pool(name="kern", bufs=1))
    sp = ctx.enter_context(tc.tile_pool(name="search", bufs=4))
    featp = ctx.enter_context(tc.tile_pool(name="feat", bufs=1))
    tmpp = ctx.enter_context(tc.tile_pool(name="tmp", bufs=1))
    diagp = ctx.enter_context(tc.tile_pool(name="diag", bufs=6))
    xrp = ctx.enter_context(tc.tile_pool(name="xrelu", bufs=1))
    outp = ctx.enter_context(tc.tile_pool(name="outs", bufs=1))
    ps_conv = ctx.enter_context(tc.tile_pool(name="ps_conv", bufs=3, space="PSUM"))
    ps_x = ctx.enter_context(tc.tile_pool(name="ps_x", bufs=3, space="PSUM"))
    ps_hd = ctx.enter_context(tc.tile_pool(name="ps_hd", bufs=2, space="PSUM"))

    # ---- weights / constants into SBUF ----
    s0_sb = []
    for kc in range(KC):
        t = sp.tile([P, 2, 31, 32], BF16, tag="sin", name=f"sin{kc}_0")
        nc.sync.dma_start(t[:], d["s_in"].ap()[kc, :, 0:2])
        s0_sb.append(t)
    csw_sb, ckw_sb, h1w_sb, h2w_sb = [], [], [], []
    csb_sb, ckb_sb, h1b_sb = [], [], []
    for kc in range(KC):
        t = wp.tile([P, 9 * 2 * P], BF16, tag=f"csw{kc}")
        nc.sync.dma_start(t[:], d["csw"].ap()[kc])
        csw_sb.append(t)
    k_sb = []
    for kc in range(KC):
        t = kp.tile([P, BPC, 9, 9], BF16, tag=f"kin{kc}")
        nc.sync.dma_start(t[:], d["k_in"].ap()[kc])
        k_sb.append(t)
    for kc in range(KC):
        t = wp.tile([P, 9 * 2 * P], BF16, tag=f"ckw{kc}")
        nc.sync.dma_start(t[:], d["ckw"].ap()[kc])
        ckw_sb.append(t)
    for kc in range(KC):
        t = wp.tile([P, 2 * P], F16, tag=f"h1w{kc}")
        nc.sync.dma_start(t[:], d["h1w"].ap()[kc])
        h1w_sb.append(t)
        t = wp.tile([P, OUT], F16, tag=f"h2w{kc}")
        nc.sync.dma_start(t[:], d["h2w"].ap()[kc])
        h2w_sb.append(t)
    for mc in range(KC):
        t = wp.tile([P, 1], F32, tag=f"csb{mc}")
        nc.sync.dma_start(t[:], d["cs_bias"].ap()[mc])
        csb_sb.append(t)
        t = wp.tile([P, 1], F32, tag=f"ckb{mc}")
        nc.sync.dma_start(t[:], d["ck_bias"].ap()[mc])
        ckb_sb.append(t)
        t = wp.tile([P, 1], F32, tag=f"h1b{mc}")
        nc.sync.dma_start(t[:], d["h1_bias"].ap()[mc])
        h1b_sb.append(t)
    h2b_sb = wp.tile([OUT, 1], F32, tag="h2b")
    nc.sync.dma_start(h2b_sb[:], d["h2_bias"].ap())
    ident_sb = wp.tile([P, P], BF16, tag="ident")
    nc.sync.dma_start(ident_sb[:], d["ident"].ap())

    # ---- persistent activations ----
    krelu_sb = [krp.tile([P, BPC * 25], F32, tag=f"krelu{mc}", name=f"krelu{mc}")
                for mc in range(KC)]
    k2_sb = [krp.tile([P, BPC * 50], BF16, tag=f"k2_{mc}", name=f"k2_{mc}")
             for mc in range(KC)]
    srelu_sb = [srp.tile([P, BPC, 29, 30], BF16, tag=f"srelu{mc}", name=f"srelu{mc}")
                for mc in range(KC)]
    for mc in range(KC):
        # zero the padding column so 746-span chains stay NaN-free
        nc.vector.memset(srelu_sb[mc][:, :, :, 29:30], 0.0)
    out_sb = outp.tile([OUT, BPC * 625], F32, tag="osb")
    ft_ring = [featp.tile([P, 750], F16, tag=f"ftr{i}", name=f"ftr{i}")
               for i in range(10)]
    for t in ft_ring:
        # zero the junk col 25..29 strips once (t-pair copies never write
        # them; fp16 head matmuls read col 25; chains keep them finite)
        nc.vector.memset(bass.AP(t[:].tensor, t[:].offset + 25,
                                 [t[:].ap[0], [30, 25], [1, 5]]), 0.0)
    tmp_ring = [tmpp.tile([P, 746], F16, tag=f"tmp{i}", name=f"tmp{i}")
                for i in range(6)]
    xr_ring = [xrp.tile([P, 626], F16, tag=f"xrr{i}", name=f"xrr{i}")
               for i in range(4)]
    for t in xr_ring:
        nc.vector.memset(t[:, 625:626], 0.0)

    def ksc(cc, b, tap):
        return krelu_sb[cc][:, b * 25 + tap:b * 25 + tap + 1]

    def win(cc, b, tap, r0=0, nr=25):
        dy, dx = tap // 5, tap % 5
        return srelu_sb[cc][:, b, dy + r0:dy + r0 + nr, dx:dx + 25]

    def win746(cc, b, tap):
        # contiguous 746-elem span of the (dy,dx)-shifted window
        dy, dx = tap // 5, tap % 5
        flat = srelu_sb[cc][:].rearrange("p b y x -> p (b y x)")
        off = b * 870 + dy * 30 + dx
        return flat[:, off:off + 746]

    def ftv(ft):
        # [25, 25] view (stride-30 rows) of the [750] fp16 feat tile
        return ft[:].rearrange("p (y x) -> p y x", x=30)[:, :, 0:25]

    def ftwin(ft, r0, nr):
        # [nr, 26] even-width window (junk col 25) for the head matmuls
        return ft[:].rearrange("p (y x) -> p y x", x=30)[:, r0:r0 + nr, 0:26]

    dg_tiles = {}

    def emit_diags(b):
        # one DVE tensor_tensor per 't'/'s' pair builds all diag(k_tap)
        # blocks: dg[p, t*128 + j] = ident[p, j] * k2[p, b*50 + 2t + (j&1)]
        for cc in range(KC):
            eng = XC_ENGINE[b * 2 + cc]
            if eng not in ("t", "s"):
                continue
            n_taps = SPLIT_TAPS if eng == "s" else 25
            dg = diagp.tile([P, 25 * P], BF16, tag="diag", name=f"dg{b}_{cc}")
            dg_tiles[b * 2 + cc] = dg
            in0 = _bcast_ap(ident_sb[:], [[0, n_taps], [2, 64], [1, 2]])
            in1 = _bcast_ap(k2_sb[cc][:, b * 50:(b + 1) * 50],
                            [[2, n_taps], [0, 64], [1, 2]])
            outv = _bcast_ap(dg[:], [[P, n_taps], [2, 64], [1, 2]])
            nc.vector.tensor_tensor(outv, in0, in1, OP.mult)

    feat_store = {}
    tmp_idx = [0]

    def emit_xcorr(b):
        # t/s pairs emit first so PE work is queued ahead of DVE chains
        for cc in sorted(range(KC),
                         key=lambda c: 0 if XC_ENGINE[b * 2 + c] in ("t", "s") else 1):
            eng = XC_ENGINE[b * 2 + cc]
            ft = ft_ring[(b * 2 + cc) % 10]
            if eng in ("t", "s"):
                pe_taps = SPLIT_TAPS if eng == "s" else 25
                ps_a = ps_x.tile([P, 13, 25], F32, tag="psx")
                ps_b = ps_x.tile([P, 12, 25], F32, tag="psx")
                dg = dg_tiles.pop(b * 2 + cc)
                for tap in range(pe_taps):
                    dgt = dg[:, tap * P:(tap + 1) * P]
                    nc.tensor.matmul(ps_a[:], dgt, win(cc, b, tap, 0, 13),
                                     start=(tap == 0), stop=(tap == pe_taps - 1))
                    nc.tensor.matmul(ps_b[:], dgt, win(cc, b, tap, 13, 12),
                                     start=(tap == 0), stop=(tap == pe_taps - 1))
                fv = ftv(ft)
                nc.scalar.activation(fv[:, 0:13, :], ps_a[:], AF.Copy)
                nc.scalar.activation(fv[:, 13:25, :], ps_b[:], AF.Copy)
                for tap in range(pe_taps, 25):
                    nc.vector.scalar_tensor_tensor(
                        ft[:, 0:746], win746(cc, b, tap), ksc(cc, b, tap),
                        ft[:, 0:746], OP.mult, OP.add)
            elif eng == "m":
                prev = None
                for tap in range(25):
                    tm = tmp_ring[tmp_idx[0] % 6]
                    tmp_idx[0] += 1
                    nc.scalar.activation(tm[:], win746(cc, b, tap), AF.Copy,
                                         scale=ksc(cc, b, tap))
                    if tap == 1:
                        nc.vector.tensor_tensor(ft[:, 0:746], prev[:], tm[:],
                                                OP.add)
                    elif tap >= 2:
                        nc.vector.tensor_tensor(ft[:, 0:746], ft[:, 0:746],
                                                tm[:], OP.add)
                    prev = tm
            else:  # 'v'
                nc.scalar.activation(ft[:, 0:746], win746(cc, b, 0), AF.Copy,
                                     scale=ksc(cc, b, 0))
                for tap in range(1, 25):
                    nc.vector.scalar_tensor_tensor(
                        ft[:, 0:746], win746(cc, b, tap), ksc(cc, b, tap),
                        ft[:, 0:746], OP.mult, OP.add)
            feat_store.setdefault(b, [None] * KC)[cc] = ft

    def emit_head(b):
        feat = feat_store.pop(b)
        xr = [xr_ring[(b * KC + mc) % 4] for mc in range(KC)]
        for mc in range(KC):
            for r0, nr in ((0, 13), (13, 12)):
                ph = ps_hd.tile([P, nr, 26], F32, tag="pshd")
                for kc in range(KC):
                    lhsT = h1w_sb[kc][:, mc * P:(mc + 1) * P]
                    nc.tensor.matmul(ph[:], lhsT, ftwin(feat[kc], r0, nr),
                                     start=(kc == 0), stop=(kc == 1))
                nc.scalar.activation(xr[mc][:, r0 * 25:(r0 + nr) * 25],
                                     ph[:, :, 0:25], AF.Relu, bias=h1b_sb[mc][:])
        for o0, n, nv in ((0, 320, 320), (320, 306, 305)):
            po = ps_hd.tile([OUT, n], F32, tag="pshd")
            for kc in range(KC):
                nc.tensor.matmul(po[:], h2w_sb[kc][:],
                                 xr[kc][:, o0:o0 + n],
                                 start=(kc == 0), stop=(kc == 1))
            nc.scalar.activation(out_sb[:, b * 625 + o0:b * 625 + o0 + nv],
                                 po[:, 0:nv], AF.Identity, bias=h2b_sb[:])

    NG = BPC // 2  # 2-sample conv groups
    for g in range(NG):
        b0 = 2 * g
        if g == 0:
            s_sb = s0_sb
        else:
            s_sb = []
            for kc in range(KC):
                t = sp.tile([P, 2, 31, 32], BF16, tag="sin", name=f"sin{kc}_{g}")
                nc.sync.dma_start(t[:], d["s_in"].ap()[kc, :, b0:b0 + 2])
                s_sb.append(t)
        for bb in (b0, b0 + 1):
            for mc in range(KC):
                for y0, nr in ((0, 15), (15, 14)):
                    pss = ps_conv.tile([P, nr, 29], F32, tag="pss")
                    i = 0
                    for tap in range(9):
                        dy, dx = tap // 3, tap % 3
                        for kc in range(KC):
                            lhsT = csw_sb[kc][:, tap * 2 * P + mc * P:tap * 2 * P + (mc + 1) * P]
                            rhs = s_sb[kc][:, bb - b0, y0 + dy:y0 + dy + nr, dx:dx + 29]
                            nc.tensor.matmul(pss[:], lhsT, rhs,
                                             start=(i == 0), stop=(i == 17))
                            i += 1
                    nc.scalar.activation(srelu_sb[mc][:, bb, y0:y0 + nr, 0:29],
                                         pss[:], AF.Relu, bias=csb_sb[mc][:])

        if g == 0:
            # conv_kernel (3x3, BN+ReLU folded): krelu[mc] = [128, b*25+tap]
            for mc in range(KC):
                psk = ps_conv.tile([P, BPC, 5, 5], F32, tag="pss")
                i = 0
                for tap in range(9):
                    dy, dx = tap // 3, tap % 3
                    for kc in range(KC):
                        lhsT = ckw_sb[kc][:, tap * 2 * P + mc * P:tap * 2 * P + (mc + 1) * P]
                        rhs = k_sb[kc][:, :, dy:dy + 5, dx:dx + 5]
                        nc.tensor.matmul(psk[:], lhsT, rhs, start=(i == 0), stop=(i == 17))
                        i += 1
                nc.scalar.activation(krelu_sb[mc][:], psk[:], AF.Relu,
                                     bias=ckb_sb[mc][:])
                # doubled bf16 copy (k2[p, 2t] = k2[p, 2t+1] = krelu[p, t])
                # for the broadcast-AP diag builds
                k2out = _bcast_ap(k2_sb[mc][:], [[2, BPC * 25], [1, 2]])
                k2in = _bcast_ap(psk[:].rearrange("p b y x -> p (b y x)"),
                                 [[1, BPC * 25], [0, 2]])
                nc.scalar.activation(k2out, k2in, AF.Relu, bias=ckb_sb[mc][:])
            emit_diags(0)
            emit_diags(1)
        if g + 1 < NG:
            emit_diags(2 * g + 2)
            emit_diags(2 * g + 3)
        if g >= 1:
            emit_xcorr(b0 - 2)
            emit_xcorr(b0 - 1)
        if g >= 2:
            emit_head(b0 - 4)
            emit_head(b0 - 3)

    emit_xcorr(BPC - 2)
    emit_xcorr(BPC - 1)
    for b in range(BPC - 4, BPC):
        emit_head(b)

    nc.sync.dma_start(d["out"].ap(), out_sb[:])


def _build_program():
    if "nc" in _prog_cache:
        return _prog_cache["nc"]
    nc = bacc.Bacc("TRN2", target_bir_lowering=False, debug=False,
                   num_devices=NCORES)
    d = {}
    d["s_in"] = nc.dram_tensor("s_in", [KC, P, BPC, 31, 32], BF16, kind="ExternalInput")
    d["k_in"] = nc.dram_tensor("k_in", [KC, P, BPC, 9, 9], BF16, kind="ExternalInput")
    d["csw"] = nc.dram_tensor("csw", [KC, P, 9, 2, P], BF16, kind="ExternalInput")
    d["ckw"] = nc.dram_tensor("ckw", [KC, P, 9, 2, P], BF16, kind="ExternalInput")
    d["cs_bias"] = nc.dram_tensor("cs_bias", [KC, P, 1], F32, kind="ExternalInput")
    d["ck_bias"] = nc.dram_tensor("ck_bias", [KC, P, 1], F32, kind="ExternalInput")
    d["h1w"] = nc.dram_tensor("h1w", [KC, P, 2, P], F16, kind="ExternalInput")
    d["h1_bias"] = nc.dram_tensor("h1_bias", [KC, P, 1], F32, kind="ExternalInput")
    d["h2w"] = nc.dram_tensor("h2w", [KC, P, OUT], F16, kind="ExternalInput")
    d["h2_bias"] = nc.dram_tensor("h2_bias", [OUT, 1], F32, kind="ExternalInput")
    d["ident"] = nc.dram_tensor("ident", [P, P], BF16, kind="ExternalInput")
    d["out"] = nc.dram_tensor("out", [OUT, BPC * 625], F32, kind="ExternalOutput")

    with tile.TileContext(nc) as tc:
        with ExitStack() as ctx:
            _emit(nc, tc, ctx, d)
    nc.compile()
    _prog_cache["nc"] = nc
    return nc


def kernel(**inputs):
    global LAST_RESULTS
    f32 = lambda x: np.ascontiguousarray(np.asarray(x), dtype=np.float32)
    kern, search = f32(inputs["kernel"]), f32(inputs["search"])

    # fold BN into conv weights / biases
    cks = f32(inputs["ck_g"]) / np.sqrt(f32(inputs["ck_v"]) + EPS)
    ckw_f = f32(inputs["ck_w"]) * cks[:, None, None, None]
    ckb = f32(inputs["ck_b"]) - f32(inputs["ck_m"]) * cks
    css = f32(inputs["cs_g"]) / np.sqrt(f32(inputs["cs_v"]) + EPS)
    csw_f = f32(inputs["cs_w"]) * css[:, None, None, None]
    csb = f32(inputs["cs_b"]) - f32(inputs["cs_m"]) * css
    h1s = f32(inputs["h_g"]) / np.sqrt(f32(inputs["h_v"]) + EPS)
    h1w_f = f32(inputs["h1_w"]) * h1s[:, None]
    h1b = f32(inputs["h_b"]) - f32(inputs["h_m"]) * h1s

    shared = {
        "csw": np.ascontiguousarray(
            csw_f.transpose(1, 2, 3, 0).reshape(KC, P, 9, 2, P)).astype(ml_dtypes.bfloat16),
        "ckw": np.ascontiguousarray(
            ckw_f.transpose(1, 2, 3, 0).reshape(KC, P, 9, 2, P)).astype(ml_dtypes.bfloat16),
        "cs_bias": csb.reshape(KC, P, 1),
        "ck_bias": ckb.reshape(KC, P, 1),
        "h1w": np.ascontiguousarray(
            h1w_f.transpose(1, 0).reshape(KC, P, 2, P)).astype(np.float16),
        "h1_bias": h1b.reshape(KC, P, 1),
        "h2w": np.ascontiguousarray(
            f32(inputs["h2_w"]).transpose(1, 0).reshape(KC, P, OUT)).astype(np.float16),
        "h2_bias": f32(inputs["h2_b"]).reshape(OUT, 1),
        "ident": np.eye(P, dtype=ml_dtypes.bfloat16),
    }
    in_maps = []
    for i in range(NCORES):
        sl = slice(i * BPC, (i + 1) * BPC)
        m = dict(shared)
        s_pad = np.zeros((KC, P, BPC, 31, 32), ml_dtypes.bfloat16)
        s_pad[..., :31] = search[sl].transpose(1, 0, 2, 3).reshape(KC, P, BPC, 31, 31)
        m["s_in"] = s_pad
        k_pad = np.zeros((KC, P, BPC, 9, 9), ml_dtypes.bfloat16)
        k_pad[..., :7, :7] = kern[sl].transpose(1, 0, 2, 3).reshape(KC, P, BPC, 7, 7)
        m["k_in"] = k_pad
        in_maps.append(m)

    nc = _build_program()
    res = run_bass_kernel_spmd(nc, in_maps, core_ids=list(range(NCORES)))
    LAST_RESULTS = res
    out = np.empty((B, OUT, 25, 25), dtype=np.float32)
    for i in range(NCORES):
        o = res.results[i]["out"].reshape(OUT, BPC, 25, 25)
        out[i * BPC:(i + 1) * BPC] = o.transpose(1, 0, 2, 3)
    return out


# revision 10
# speedup vs baseline: 1.1740x; 1.1740x over previous
"""Trainium2 Bass kernel for DepthwiseXCorr (SiamRPN-style) model.

Pipeline (per sample): conv3x3+BN+ReLU on kernel & search branches,
depthwise cross-correlation, 1x1 conv + BN + ReLU head, 1x1 conv + bias.

Sharding: data-parallel over batch across 8 NeuronCores (8 samples each),
weights replicated.  BN is folded into conv weights on the host.

Layout on device: channels on SBUF partitions (2 chunks of 128), spatial x
batch on the free dimension.  The search conv runs as 9 shifted bf16
matmuls over 2-sample groups (halves the instruction count).  The
depthwise xcorr is distributed per (sample, channel-chunk) pair across
engines: 't' = TensorE diag(k_tap) matmuls accumulated in PSUM (the 25
diagonals of a pair are built by ONE VectorE tensor_tensor instruction
with broadcast access patterns, 2x 16-bit mode); 'v' = VectorE
scalar_tensor_tensor chains over contiguous 746-elem spans (first tap on
ScalarE); 'm' = ScalarE produces the 25 scaled tap products and VectorE
folds them with 2x fp16 tensor_tensor adds; 's' = split TensorE/VectorE.
Feat/head tensors are fp16 (validated 3.2e-3 end-to-end).  The engine
pipeline is staggered: conv(group g) || xcorr(group g-1) || heads(g-2).
"""

import sys

if "/opt/trn_rl_repo" not in sys.path:
    sys.path.insert(0, "/opt/trn_rl_repo")

from contextlib import ExitStack

import ml_dtypes
import numpy as np

import concourse.bass as bass
import concourse.tile as tile
from concourse import bacc, mybir
from concourse.bass_utils import run_bass_kernel_spmd

EPS = 1e-5
NCORES = 8
B, C, HID, OUT = 64, 256, 256, 10
BPC = B // NCORES  # samples per core
P = 128
KC = C // P  # channel chunks (2)
F32 = mybir.dt.float32
F16 = mybir.dt.float16
BF16 = mybir.dt.bfloat16
AF = mybir.ActivationFunctionType
OP = mybir.AluOpType

# xcorr engine per (b, cc) pair, index p = b*2 + cc
# 't' TensorE diag-matmul | 'v' VectorE STT chain | 'm' ScalarE products +
# VectorE 2x adds | 's' split (SPLIT_TAPS on TensorE, rest on VectorE)
XC_ENGINE = ["s", "m", "m", "v", "v", "m", "m", "v",
             "t", "m", "v", "t", "t", "t", "t", "t"]
SPLIT_TAPS = 15

LAST_RESULTS = None  # BassKernelResults of the most recent run (for profiling)

_prog_cache = {}


def _bcast_ap(base, dims):
    """Raw AP with the tile's partition dim plus explicit [stride, count] dims."""
    return bass.AP(base.tensor, base.offset, [base.ap[0]] + dims)


def _emit(nc, tc, ctx, d):
    """Emit the per-core program.  d maps dram tensor name -> handle."""
    wp = ctx.enter_context(tc.tile_pool(name="weights", bufs=1))
    srp = ctx.enter_context(tc.tile_pool(name="srelu", bufs=1))
    krp = ctx.enter_context(tc.tile_pool(name="krelu", bufs=1))
    kp = ctx.enter_context(tc.tile_pool(name="kern", bufs=1))
    sp = ctx.enter_context(tc.tile_pool(name="search", bufs=4))
    featp = ctx.enter_context(tc.tile_pool(name="feat", bufs=1))
    tmpp = ctx.enter_context(tc.tile_pool(name="tmp", bufs=1))
    diagp = ctx.enter_context(tc.tile_pool(name="diag", bufs=6))
    xrp = ctx.enter_context(tc.tile_pool(name="xrelu", bufs=1))
    outp = ctx.enter_context(tc.tile_pool(name="outs", bufs=1))
    ps_conv = ctx.enter_context(tc.tile_pool(name="ps_conv", bufs=3, space="PSUM"))
    ps_x = ctx.enter_context(tc.tile_pool(name="ps_x", bufs=3, space="PSUM"))
    ps_hd = ctx.enter_context(tc.tile_pool(name="ps_hd", bufs=2, space="PSUM"))

    # ---- weights / constants into SBUF ----
    s0_sb = []
    for kc in range(KC):
        t = sp.tile([P, 2, 31, 32], BF16, tag="sin", name=f"sin{kc}_0")
        nc.sync.dma_start(t[:], d["s_in"].ap()[kc, :, 0:2])
        s0_sb.append(t)
    csw_sb, ckw_sb, h1w_sb, h2w_sb = [], [], [], []
    csb_sb, ckb_sb, h1b_sb = [], [], []
    for kc in range(KC):
        t = wp.tile([P, 9 * 2 * P], BF16, tag=f"csw{kc}")
        nc.sync.dma_start(t[:], d["csw"].ap()[kc])
        csw_sb.append(t)
    k_sb = []
    for kc in range(KC):
        t = kp.tile([P, BPC, 9, 9], BF16, tag=f"kin{kc}")
        nc.sync.dma_start(t[:], d["k_in"].ap()[kc])
        k_sb.append(t)
    for kc in range(KC):
        t = wp.tile([P, 9 * 2 * P], BF16, tag=f"ckw{kc}")
        nc.sync.dma_start(t[:], d["ckw"].ap()[kc])
        ckw_sb.append(t)
    for kc in range(KC):
        t = wp.tile([P, 2 * P], F16, tag=f"h1w{kc}")
        nc.sync.dma_start(t[:], d["h1w"].ap()[kc])
        h1w_sb.append(t)
        t = wp.tile([P, OUT], F16, tag=f"h2w{kc}")
        nc.sync.dma_start(t[:], d["h2w"].ap()[kc])
        h2w_sb.append(t)
    for mc in range(KC):
        t = wp.tile([P, 1], F32, tag=f"csb{mc}")
        nc.sync.dma_start(t[:], d["cs_bias"].ap()[mc])
        csb_sb.append(t)
        t = wp.tile([P, 1], F32, tag=f"ckb{mc}")
        nc.sync.dma_start(t[:], d["ck_bias"].ap()[mc])
        ckb_sb.append(t)
        t = wp.tile([P, 1], F32, tag=f"h1b{mc}")
        nc.sync.dma_start(t[:], d["h1_bias"].ap()[mc])
        h1b_sb.append(t)
    h2b_sb = wp.tile([OUT, 1], F32, tag="h2b")
    nc.sync.dma_start(h2b_sb[:], d["h2_bias"].ap())
    ident_sb = wp.tile([P, P], BF16, tag="ident")
    nc.sync.dma_start(ident_sb[:], d["ident"].ap())

    # ---- persistent activations ----
    krelu_sb = [krp.tile([P, BPC * 25], F32, tag=f"krelu{mc}", name=f"krelu{mc}")
                for mc in range(KC)]
    k2_sb = [krp.tile([P, BPC * 50], BF16, tag=f"k2_{mc}", name=f"k2_{mc}")
             for mc in range(KC)]
    srelu_sb = [srp.tile([P, BPC, 29, 30], BF16, tag=f"srelu{mc}", name=f"srelu{mc}")
                for mc in range(KC)]
    for mc in range(KC):
        # zero the padding column so 746-span chains stay NaN-free
        nc.vector.memset(srelu_sb[mc][:, :, :, 29:30], 0.0)
    out_sb = outp.tile([OUT, BPC * 625], F32, tag="osb")
    ft_ring = [featp.tile([P, 750], F16, tag=f"ftr{i}", name=f"ftr{i}")
               for i in range(10)]
    for t in ft_ring:
        # zero the junk col 25..29 strips once (t-pair copies never write
        # them; fp16 head matmuls read col 25; chains keep them finite)
        nc.vector.memset(bass.AP(t[:].tensor, t[:].offset + 25,
                                 [t[:].ap[0], [30, 25], [1, 5]]), 0.0)
    tmp_ring = [tmpp.tile([P, 746], F16, tag=f"tmp{i}", name=f"tmp{i}")
                for i in range(6)]
    xr_ring = [xrp.tile([P, 626], F16, tag=f"xrr{i}", name=f"xrr{i}")
               for i in range(4)]
    for t in xr_ring:
        nc.vector.memset(t[:, 625:626], 0.0)

    def ksc(cc, b, tap):
        return krelu_sb[cc][:, b * 25 + tap:b * 25 + tap + 1]

    def win(cc, b, tap, r0=0, nr=25):
        dy, dx = tap // 5, tap % 5
        return srelu_sb[cc][:, b, dy + r0:dy + r0 + nr, dx:dx + 25]

    def win746(cc, b, tap):
        # contiguous 746-elem span of the (dy,dx)-shifted window
        dy, dx = tap // 5, tap % 5
        flat = srelu_sb[cc][:].rearrange("p b y x -> p (b y x)")
        off = b * 870 + dy * 30 + dx
        return flat[:, off:off + 746]

    def ftv(ft):
        # [25, 25] view (stride-30 rows) of the [750] fp16 feat tile
        return ft[:].rearrange("p (y x) -> p y x", x=30)[:, :, 0:25]

    def ftwin(ft, r0, nr):
        # [nr, 26] even-width window (junk col 25) for the head matmuls
        return ft[:].rearrange("p (y x) -> p y x", x=30)[:, r0:r0 + nr, 0:26]

    dg_tiles = {}

    def emit_diags(b):
        # one DVE tensor_tensor per 't'/'s' pair builds all diag(k_tap)
        # blocks: dg[p, t*128 + j] = ident[p, j] * k2[p, b*50 + 2t + (j&1)]
        for cc in range(KC):
            eng = XC_ENGINE[b * 2 + cc]
            if eng not in ("t", "s"):
                continue
            n_taps = SPLIT_TAPS if eng == "s" else 25
            dg = diagp.tile([P, 25 * P], BF16, tag="diag", name=f"dg{b}_{cc}")
            dg_tiles[b * 2 + cc] = dg
            in0 = _bcast_ap(ident_sb[:], [[0, n_taps], [2, 64], [1, 2]])
            in1 = _bcast_ap(k2_sb[cc][:, b * 50:(b + 1) * 50],
                            [[2, n_taps], [0, 64], [1, 2]])
            outv = _bcast_ap(dg[:], [[P, n_taps], [2, 64], [1, 2]])
            nc.vector.tensor_tensor(outv, in0, in1, OP.mult)

    feat_store = {}
    tmp_idx = [0]

    def emit_xcorr(b):
        # t/s pairs emit first so PE work is queued ahead of DVE chains
        for cc in sorted(range(KC),
                         key=lambda c: 0 if XC_ENGINE[b * 2 + c] in ("t", "s") else 1):
            eng = XC_ENGINE[b * 2 + cc]
            ft = ft_ring[(b * 2 + cc) % 10]
            if eng in ("t", "s"):
                pe_taps = SPLIT_TAPS if eng == "s" else 25
                ps_a = ps_x.tile([P, 13, 25], F32, tag="psx")
                ps_b = ps_x.tile([P, 12, 25], F32, tag="psx")
                dg = dg_tiles.pop(b * 2 + cc)
                for tap in range(pe_taps):
                    dgt = dg[:, tap * P:(tap + 1) * P]
                    nc.tensor.matmul(ps_a[:], dgt, win(cc, b, tap, 0, 13),
                                     start=(tap == 0), stop=(tap == pe_taps - 1))
                    nc.tensor.matmul(ps_b[:], dgt, win(cc, b, tap, 13, 12),
                                     start=(tap == 0), stop=(tap == pe_taps - 1))
                fv = ftv(ft)
                nc.scalar.activation(fv[:, 0:13, :], ps_a[:], AF.Copy)
                nc.scalar.activation(fv[:, 13:25, :], ps_b[:], AF.Copy)
                for tap in range(pe_taps, 25):
                    nc.vector.scalar_tensor_tensor(
                        ft[:, 0:746], win746(cc, b, tap), ksc(cc, b, tap),
                        ft[:, 0:746], OP.mult, OP.add)
            elif eng == "m":
                prev = None
                for tap in range(25):
                    tm = tmp_ring[tmp_idx[0] % 6]
                    tmp_idx[0] += 1
                    nc.scalar.activation(tm[:], win746(cc, b, tap), AF.Copy,
                                         scale=ksc(cc, b, tap))
                    if tap == 1:
                        nc.vector.tensor_tensor(ft[:, 0:746], prev[:], tm[:],
                                                OP.add)
                    elif tap >= 2:
                        nc.vector.tensor_tensor(ft[:, 0:746], ft[:, 0:746],
                                                tm[:], OP.add)
                    prev = tm
            else:  # 'v'
                nc.scalar.activation(ft[:, 0:746], win746(cc, b, 0), AF.Copy,
                                     scale=ksc(cc, b, 0))
                for tap in range(1, 25):
                    nc.vector.scalar_tensor_tensor(
                        ft[:, 0:746], win746(cc, b, tap), ksc(cc, b, tap),
                        ft[:, 0:746], OP.mult, OP.add)
            feat_store.setdefault(b, [None] * KC)[cc] = ft

    def emit_head(b):
        feat = feat_store.pop(b)
        xr = [xr_ring[(b * KC + mc) % 4] for mc in range(KC)]
        for mc in range(KC):
            for r0, nr in ((0, 13), (13, 12)):
                ph = ps_hd.tile([P, nr, 26], F32, tag="pshd")
                for kc in range(KC):
                    lhsT = h1w_sb[kc][:, mc * P:(mc + 1) * P]
                    nc.tensor.matmul(ph[:], lhsT, ftwin(feat[kc], r0, nr),
                                     start=(kc == 0), stop=(kc == 1))
                nc.scalar.activation(xr[mc][:, r0 * 25:(r0 + nr) * 25],
                                     ph[:, :, 0:25], AF.Relu, bias=h1b_sb[mc][:])
        for o0, n, nv in ((0, 320, 320), (320, 306, 305)):
            po = ps_hd.tile([OUT, n], F32, tag="pshd")
            for kc in range(KC):
                nc.tensor.matmul(po[:], h2w_sb[kc][:],
                                 xr[kc][:, o0:o0 + n],
                                 start=(kc == 0), stop=(kc == 1))
            nc.scalar.activation(out_sb[:, b * 625 + o0:b * 625 + o0 + nv],
                                 po[:, 0:nv], AF.Identity, bias=h2b_sb[:])

    NG = BPC // 2  # 2-sample conv groups
    for g in range(NG):
        b0 = 2 * g
        if g == 0:
            s_sb = s0_sb
        else:
            s_sb = []
            for kc in range(KC):
                t = sp.tile([P, 2, 31, 32], BF16, tag="sin", name=f"sin{kc}_{g}")
                nc.sync.dma_start(t[:], d["s_in"].ap()[kc, :, b0:b0 + 2])
                s_sb.append(t)
        for bb in (b0, b0 + 1):
            for mc in range(KC):
                for y0, nr in ((0, 15), (15, 14)):
                    pss = ps_conv.tile([P, nr, 29], F32, tag="pss")
                    i = 0
                    for tap in range(9):
                        dy, dx = tap // 3, tap % 3
                        for kc in range(KC):
                            lhsT = csw_sb[kc][:, tap * 2 * P + mc * P:tap * 2 * P + (mc + 1) * P]
                            rhs = s_sb[kc][:, bb - b0, y0 + dy:y0 + dy + nr, dx:dx + 29]
                            nc.tensor.matmul(pss[:], lhsT, rhs,
                                             start=(i == 0), stop=(i == 17))
                            i += 1
                    nc.scalar.activation(srelu_sb[mc][:, bb, y0:y0 + nr, 0:29],
                                         pss[:], AF.Relu, bias=csb_sb[mc][:])

        if g == 0:
            # conv_kernel (3x3, BN+ReLU folded): krelu[mc] = [128, b*25+tap]
            for mc in range(KC):
                psk = ps_conv.tile([P, BPC, 5, 5], F32, tag="pss")
                i = 0
                for tap in range(9):
                    dy, dx = tap // 3, tap % 3
                    for kc in range(KC):
                        lhsT = ckw_sb[kc][:, tap * 2 * P + mc * P:tap * 2 * P + (mc + 1) * P]
                        rhs = k_sb[kc][:, :, dy:dy + 5, dx:dx + 5]
                        nc.tensor.matmul(psk[:], lhsT, rhs, start=(i == 0), stop=(i == 17))
                        i += 1
                nc.scalar.activation(krelu_sb[mc][:], psk[:], AF.Relu,
                                     bias=ckb_sb[mc][:])
                # doubled bf16 copy (k2[p, 2t] = k2[p, 2t+1] = krelu[p, t])
                # for the broadcast-AP diag builds
                k2out = _bcast_ap(k2_sb[mc][:], [[2, BPC * 25], [1, 2]])
                k2in = _bcast_ap(psk[:].rearrange("p b y x -> p (b y x)"),
                                 [[1, BPC * 25], [0, 2]])
                nc.scalar.activation(k2out, k2in, AF.Relu, bias=ckb_sb[mc][:])
            emit_diags(0)
            emit_diags(1)
        if g + 1 < NG:
            emit_diags(2 * g + 2)
            emit_diags(2 * g + 3)
        # heads BEFORE this iteration's xcorr chains: their ScalarE ops must
        # not queue behind ~40us of 'm'-pair copies or ps_hd backs up PE
        if g >= 2:
            emit_head(b0 - 4)
            emit_head(b0 - 3)
        if g >= 1:
            emit_xcorr(b0 - 2)
            emit_xcorr(b0 - 1)

    emit_head(BPC - 4)
    emit_head(BPC - 3)
    emit_xcorr(BPC - 2)
    emit_xcorr(BPC - 1)
    emit_head(BPC - 2)
    emit_head(BPC - 1)

    nc.sync.dma_start(d["out"].ap(), out_sb[:])


def _build_program():
    if "nc" in _prog_cache:
        return _prog_cache["nc"]
    nc = bacc.Bacc("TRN2", target_bir_lowering=False, debug=False,
                   num_devices=NCORES)
    d = {}
    d["s_in"] = nc.dram_tensor("s_in", [KC, P, BPC, 31, 32], BF16, kind="ExternalInput")
    d["k_in"] = nc.dram_tensor("k_in", [KC, P, BPC, 9, 9], BF16, kind="ExternalInput")
    d["csw"] = nc.dram_tensor("csw", [KC, P, 9, 2, P], BF16, kind="ExternalInput")
    d["ckw"] = nc.dram_tensor("ckw", [KC, P, 9, 2, P], BF16, kind="ExternalInput")
    d["cs_bias"] = nc.dram_tensor("cs_bias", [KC, P, 1], F32, kind="ExternalInput")
    d["ck_bias"] = nc.dram_tensor("ck_bias", [KC, P, 1], F32, kind="ExternalInput")
    d["h1w"] = nc.dram_tensor("h1w", [KC, P, 2, P], F16, kind="ExternalInput")
    d["h1_bias"] = nc.dram_tensor("h1_bias", [KC, P, 1], F32, kind="ExternalInput")
    d["h2w"] = nc.dram_tensor("h2w", [KC, P, OUT], F16, kind="ExternalInput")
    d["h2_bias"] = nc.dram_tensor("h2_bias", [OUT, 1], F32, kind="ExternalInput")
    d["ident"] = nc.dram_tensor("ident", [P, P], BF16, kind="ExternalInput")
    d["out"] = nc.dram_tensor("out", [OUT, BPC * 625], F32, kind="ExternalOutput")

    with tile.TileContext(nc) as tc:
        with ExitStack() as ctx:
            _emit(nc, tc, ctx, d)
    nc.compile()
    _prog_cache["nc"] = nc
    return nc


def kernel(**inputs):
    global LAST_RESULTS
    f32 = lambda x: np.ascontiguousarray(np.asarray(x), dtype=np.float32)
    kern, search = f32(inputs["kernel"]), f32(inputs["search"])

    # fold BN into conv weights / biases
    cks = f32(inputs["ck_g"]) / np.sqrt(f32(inputs["ck_v"]) + EPS)
    ckw_f = f32(inputs["ck_w"]) * cks[:, None, None, None]
    ckb = f32(inputs["ck_b"]) - f32(inputs["ck_m"]) * cks
    css = f32(inputs["cs_g"]) / np.sqrt(f32(inputs["cs_v"]) + EPS)
    csw_f = f32(inputs["cs_w"]) * css[:, None, None, None]
    csb = f32(inputs["cs_b"]) - f32(inputs["cs_m"]) * css
    h1s = f32(inputs["h_g"]) / np.sqrt(f32(inputs["h_v"]) + EPS)
    h1w_f = f32(inputs["h1_w"]) * h1s[:, None]
    h1b = f32(inputs["h_b"]) - f32(inputs["h_m"]) * h1s

    shared = {
        "csw": np.ascontiguousarray(
            csw_f.transpose(1, 2, 3, 0).reshape(KC, P, 9, 2, P)).astype(ml_dtypes.bfloat16),
        "ckw": np.ascontiguousarray(
            ckw_f.transpose(1, 2, 3, 0).reshape(KC, P, 9, 2, P)).astype(ml_dtypes.bfloat16),
        "cs_bias": csb.reshape(KC, P, 1),
        "ck_bias": ckb.reshape(KC, P, 1),
        "h1w": np.ascontiguousarray(
            h1w_f.transpose(1, 0).reshape(KC, P, 2, P)).astype(np.float16),
        "h1_bias": h1b.reshape(KC, P, 1),
        "h2w": np.ascontiguousarray(
            f32(inputs["h2_w"]).transpose(1, 0).reshape(KC, P, OUT)).astype(np.float16),
        "h2_bias": f32(inputs["h2_b"]).reshape(OUT, 1),
        "ident": np.eye(P, dtype=ml_dtypes.bfloat16),
    }
    in_maps = []
    for i in range(NCORES):
        sl = slice(i * BPC, (i + 1) * BPC)
        m = dict(shared)
        s_pad = np.zeros((KC, P, BPC, 31, 32), ml_dtypes.bfloat16)
        s_pad[..., :31] = search[sl].transpose(1, 0, 2, 3).reshape(KC, P, BPC, 31, 31)
        m["s_in"] = s_pad
        k_pad = np.zeros((KC, P, BPC, 9, 9), ml_dtypes.bfloat16)
        k_pad[..., :7, :7] = kern[sl].transpose(1, 0, 2, 3).reshape(KC, P, BPC, 7, 7)
        m["k_in"] = k_pad
        in_maps.append(m)

    nc = _build_program()
    res = run_bass_kernel_spmd(nc, in_maps, core_ids=list(range(NCORES)))
    LAST_RESULTS = res
    out = np.empty((B, OUT, 25, 25), dtype=np.float32)
    for i in range(NCORES):
        o = res.results[i]["out"].reshape(OUT, BPC, 25, 25)
        out[i * BPC:(i + 1) * BPC] = o.transpose(1, 0, 2, 3)
    return out


# revision 15
# speedup vs baseline: 1.2857x; 1.0952x over previous
"""Trainium2 Bass kernel for DepthwiseXCorr (SiamRPN-style) model.

Pipeline (per sample): conv3x3+BN+ReLU on kernel & search branches,
depthwise cross-correlation, 1x1 conv + BN + ReLU head, 1x1 conv + bias.

Sharding: data-parallel over batch across 8 NeuronCores (8 samples each),
weights replicated.  BN is folded into conv weights on the host.

Layout on device: channels on SBUF partitions (2 chunks of 128), spatial x
batch on the free dimension.  The search conv runs as 9 shifted bf16
matmuls over 2-sample groups (halves the instruction count).  The
depthwise xcorr is distributed per (sample, channel-chunk) pair across
engines: 't' = TensorE diag(k_tap) matmuls accumulated in PSUM (the 25
diagonals of a pair are built by ONE VectorE tensor_tensor instruction
with broadcast access patterns, 2x 16-bit mode); 'v' = VectorE
scalar_tensor_tensor chains over contiguous 746-elem spans (first tap on
ScalarE); 'm' = ScalarE produces the 25 scaled tap products and VectorE
folds them with 2x fp16 tensor_tensor adds; 's' = split TensorE/VectorE.
Feat/head tensors are fp16 (validated 3.2e-3 end-to-end).  The engine
pipeline is staggered: conv(group g) || xcorr(group g-1) || heads(g-2).
"""

import sys

if "/opt/trn_rl_repo" not in sys.path:
    sys.path.insert(0, "/opt/trn_rl_repo")

from contextlib import ExitStack

import ml_dtypes
import numpy as np

import concourse.bass as bass
import concourse.tile as tile
from concourse import bacc, mybir
from concourse.bass_utils import run_bass_kernel_spmd

EPS = 1e-5
NCORES = 8
B, C, HID, OUT = 64, 256, 256, 10
BPC = B // NCORES  # samples per core
P = 128
KC = C // P  # channel chunks (2)
F32 = mybir.dt.float32
F16 = mybir.dt.float16
BF16 = mybir.dt.bfloat16
AF = mybir.ActivationFunctionType
OP = mybir.AluOpType

# xcorr engine per (b, cc) pair, index p = b*2 + cc
# 't' TensorE diag-matmul | 'v' VectorE STT chain | 'm' ScalarE products +
# VectorE 2x adds | 's' split (SPLIT_TAPS on TensorE, rest on VectorE)
XC_ENGINE = ["m", "v", "v", "m", "m", "v", "v", "m",
             "t", "t", "t", "t", "t", "t", "t", "t"]
SPLIT_TAPS = 15

LAST_RESULTS = None  # BassKernelResults of the most recent run (for profiling)

_prog_cache = {}


def _bcast_ap(base, dims):
    """Raw AP with the tile's partition dim plus explicit [stride, count] dims."""
    return bass.AP(base.tensor, base.offset, [base.ap[0]] + dims)


def _emit(nc, tc, ctx, d):
    """Emit the per-core program.  d maps dram tensor name -> handle."""
    wp = ctx.enter_context(tc.tile_pool(name="weights", bufs=1))
    srp = ctx.enter_context(tc.tile_pool(name="srelu", bufs=1))
    krp = ctx.enter_context(tc.tile_pool(name="krelu", bufs=1))
    kp = ctx.enter_context(tc.tile_pool(name="kern", bufs=1))
    sp = ctx.enter_context(tc.tile_pool(name="search", bufs=4))
    featp = ctx.enter_context(tc.tile_pool(name="feat", bufs=1))
    tmpp = ctx.enter_context(tc.tile_pool(name="tmp", bufs=1))
    diagp = ctx.enter_context(tc.tile_pool(name="diag", bufs=6))
    xrp = ctx.enter_context(tc.tile_pool(name="xrelu", bufs=1))
    outp = ctx.enter_context(tc.tile_pool(name="outs", bufs=1))
    ps_conv = ctx.enter_context(tc.tile_pool(name="ps_conv", bufs=2, space="PSUM"))
    ps_x = ctx.enter_context(tc.tile_pool(name="ps_x", bufs=3, space="PSUM"))
    ps_hd = ctx.enter_context(tc.tile_pool(name="ps_hd", bufs=3, space="PSUM"))

    # ---- weights / constants into SBUF ----
    s0_sb = []
    for kc in range(KC):
        t = sp.tile([P, 2, 31, 32], BF16, tag="sin", name=f"sin{kc}_0")
        nc.sync.dma_start(t[:], d["s_in"].ap()[kc, :, 0:2])
        s0_sb.append(t)
    csw_sb, ckw_sb, h1w_sb, h2w_sb = [], [], [], []
    csb_sb, ckb_sb, h1b_sb = [], [], []
    for kc in range(KC):
        t = wp.tile([P, 9 * 2 * P], BF16, tag=f"csw{kc}")
        nc.scalar.dma_start(t[:], d["csw"].ap()[kc])
        csw_sb.append(t)
    k_sb = []
    for kc in range(KC):
        t = kp.tile([P, BPC, 9, 9], BF16, tag=f"kin{kc}")
        nc.sync.dma_start(t[:], d["k_in"].ap()[kc])
        k_sb.append(t)
    for kc in range(KC):
        t = wp.tile([P, 9 * 2 * P], BF16, tag=f"ckw{kc}")
        nc.sync.dma_start(t[:], d["ckw"].ap()[kc])
        ckw_sb.append(t)
    for kc in range(KC):
        t = wp.tile([P, 2 * P], F16, tag=f"h1w{kc}")
        nc.sync.dma_start(t[:], d["h1w"].ap()[kc])
        h1w_sb.append(t)
        t = wp.tile([P, OUT], F16, tag=f"h2w{kc}")
        nc.sync.dma_start(t[:], d["h2w"].ap()[kc])
        h2w_sb.append(t)
    for mc in range(KC):
        t = wp.tile([P, 1], F32, tag=f"csb{mc}")
        nc.sync.dma_start(t[:], d["cs_bias"].ap()[mc])
        csb_sb.append(t)
        t = wp.tile([P, 1], F32, tag=f"ckb{mc}")
        nc.sync.dma_start(t[:], d["ck_bias"].ap()[mc])
        ckb_sb.append(t)
        t = wp.tile([P, 1], F32, tag=f"h1b{mc}")
        nc.sync.dma_start(t[:], d["h1_bias"].ap()[mc])
        h1b_sb.append(t)
    h2b_sb = wp.tile([OUT, 1], F32, tag="h2b")
    nc.sync.dma_start(h2b_sb[:], d["h2_bias"].ap())
    ident_sb = wp.tile([P, P], BF16, tag="ident")
    nc.sync.dma_start(ident_sb[:], d["ident"].ap())

    # ---- persistent activations ----
    krelu_sb = [krp.tile([P, BPC * 25], F32, tag=f"krelu{mc}", name=f"krelu{mc}")
                for mc in range(KC)]
    k2_sb = [krp.tile([P, BPC * 50], BF16, tag=f"k2_{mc}", name=f"k2_{mc}")
             for mc in range(KC)]
    srelu_sb = [srp.tile([P, BPC, 29, 30], BF16, tag=f"srelu{mc}", name=f"srelu{mc}")
                for mc in range(KC)]
    for mc in range(KC):
        # zero the padding column so 746-span chains stay NaN-free
        nc.vector.memset(srelu_sb[mc][:, :, :, 29:30], 0.0)
    out_sb = outp.tile([OUT, BPC * 625], F32, tag="osb")
    ft_ring = [featp.tile([P, 750], F16, tag=f"ftr{i}", name=f"ftr{i}")
               for i in range(10)]
    for t in ft_ring:
        # zero the junk col 25..29 strips once (t-pair copies never write
        # them; fp16 head matmuls read col 25; chains keep them finite)
        nc.vector.memset(bass.AP(t[:].tensor, t[:].offset + 25,
                                 [t[:].ap[0], [30, 25], [1, 5]]), 0.0)
    tmp_ring = [tmpp.tile([P, 746], F16, tag=f"tmp{i}", name=f"tmp{i}")
                for i in range(6)]
    xr_ring = [xrp.tile([P, 626], F16, tag=f"xrr{i}", name=f"xrr{i}")
               for i in range(4)]
    for t in xr_ring:
        nc.vector.memset(t[:, 625:626], 0.0)

    def ksc(cc, b, tap):
        return krelu_sb[cc][:, b * 25 + tap:b * 25 + tap + 1]

    def win(cc, b, tap, r0=0, nr=25):
        dy, dx = tap // 5, tap % 5
        return srelu_sb[cc][:, b, dy + r0:dy + r0 + nr, dx:dx + 25]

    def win746(cc, b, tap):
        # contiguous 746-elem span of the (dy,dx)-shifted window
        dy, dx = tap // 5, tap % 5
        flat = srelu_sb[cc][:].rearrange("p b y x -> p (b y x)")
        off = b * 870 + dy * 30 + dx
        return flat[:, off:off + 746]

    def ftv(ft):
        # [25, 25] view (stride-30 rows) of the [750] fp16 feat tile
        return ft[:].rearrange("p (y x) -> p y x", x=30)[:, :, 0:25]

    def ftwin(ft, r0, nr):
        # [nr, 26] even-width window (junk col 25) for the head matmuls
        return ft[:].rearrange("p (y x) -> p y x", x=30)[:, r0:r0 + nr, 0:26]

    dg_tiles = {}

    def emit_diags(b):
        # one DVE tensor_tensor per 't'/'s' pair builds all diag(k_tap)
        # blocks: dg[p, t*128 + j] = ident[p, j] * k2[p, b*50 + 2t + (j&1)]
        for cc in range(KC):
            eng = XC_ENGINE[b * 2 + cc]
            if eng not in ("t", "s"):
                continue
            n_taps = SPLIT_TAPS if eng == "s" else 25
            dg = diagp.tile([P, 25 * P], BF16, tag="diag", name=f"dg{b}_{cc}")
            dg_tiles[b * 2 + cc] = dg
            in0 = _bcast_ap(ident_sb[:], [[0, n_taps], [2, 64], [1, 2]])
            in1 = _bcast_ap(k2_sb[cc][:, b * 50:(b + 1) * 50],
                            [[2, n_taps], [0, 64], [1, 2]])
            outv = _bcast_ap(dg[:], [[P, n_taps], [2, 64], [1, 2]])
            nc.vector.tensor_tensor(outv, in0, in1, OP.mult)

    feat_store = {}
    tmp_idx = [0]

    def emit_xcorr(b):
        # t/s pairs emit first so PE work is queued ahead of DVE chains
        for cc in sorted(range(KC),
                         key=lambda c: 0 if XC_ENGINE[b * 2 + c] in ("t", "s") else 1):
            eng = XC_ENGINE[b * 2 + cc]
            ft = ft_ring[(b * 2 + cc) % 10]
            if eng in ("t", "s"):
                pe_taps = SPLIT_TAPS if eng == "s" else 25
                ps_a = ps_x.tile([P, 13, 25], F32, tag="psx")
                ps_b = ps_x.tile([P, 12, 25], F32, tag="psx")
                dg = dg_tiles.pop(b * 2 + cc)
                for tap in range(pe_taps):
                    dgt = dg[:, tap * P:(tap + 1) * P]
                    nc.tensor.matmul(ps_a[:], dgt, win(cc, b, tap, 0, 13),
                                     start=(tap == 0), stop=(tap == pe_taps - 1))
                    nc.tensor.matmul(ps_b[:], dgt, win(cc, b, tap, 13, 12),
                                     start=(tap == 0), stop=(tap == pe_taps - 1))
                fv = ftv(ft)
                nc.scalar.activation(fv[:, 0:13, :], ps_a[:], AF.Copy)
                nc.scalar.activation(fv[:, 13:25, :], ps_b[:], AF.Copy)
                for tap in range(pe_taps, 25):
                    nc.vector.scalar_tensor_tensor(
                        ft[:, 0:746], win746(cc, b, tap), ksc(cc, b, tap),
                        ft[:, 0:746], OP.mult, OP.add)
            elif eng == "m":
                prev = None
                for tap in range(25):
                    tm = tmp_ring[tmp_idx[0] % 6]
                    tmp_idx[0] += 1
                    nc.scalar.activation(tm[:], win746(cc, b, tap), AF.Copy,
                                         scale=ksc(cc, b, tap))
                    if tap == 1:
                        nc.vector.tensor_tensor(ft[:, 0:746], prev[:], tm[:],
                                                OP.add)
                    elif tap >= 2:
                        nc.vector.tensor_tensor(ft[:, 0:746], ft[:, 0:746],
                                                tm[:], OP.add)
                    prev = tm
            else:  # 'v'
                nc.scalar.activation(ft[:, 0:746], win746(cc, b, 0), AF.Copy,
                                     scale=ksc(cc, b, 0))
                for tap in range(1, 25):
                    nc.vector.scalar_tensor_tensor(
                        ft[:, 0:746], win746(cc, b, tap), ksc(cc, b, tap),
                        ft[:, 0:746], OP.mult, OP.add)
            feat_store.setdefault(b, [None] * KC)[cc] = ft

    def emit_head(b):
        feat = feat_store.pop(b)
        xr = [xr_ring[(b * KC + mc) % 4] for mc in range(KC)]
        for mc in range(KC):
            for r0, nr in ((0, 13), (13, 12)):
                ph = ps_hd.tile([P, nr, 26], F32, tag="pshd")
                for kc in range(KC):
                    lhsT = h1w_sb[kc][:, mc * P:(mc + 1) * P]
                    nc.tensor.matmul(ph[:], lhsT, ftwin(feat[kc], r0, nr),
                                     start=(kc == 0), stop=(kc == 1))
                nc.scalar.activation(xr[mc][:, r0 * 25:(r0 + nr) * 25],
                                     ph[:, :, 0:25], AF.Relu, bias=h1b_sb[mc][:])
        for o0, n, nv in ((0, 320, 320), (320, 306, 305)):
            po = ps_hd.tile([OUT, n], F32, tag="pshd")
            for kc in range(KC):
                nc.tensor.matmul(po[:], h2w_sb[kc][:],
                                 xr[kc][:, o0:o0 + n],
                                 start=(kc == 0), stop=(kc == 1))
            nc.scalar.activation(out_sb[:, b * 625 + o0:b * 625 + o0 + nv],
                                 po[:, 0:nv], AF.Identity, bias=h2b_sb[:])

    NG = BPC // 2  # 2-sample conv groups
    for g in range(NG):
        b0 = 2 * g
        if g == 0:
            s_sb = s0_sb
        else:
            s_sb = []
            for kc in range(KC):
                t = sp.tile([P, 2, 31, 32], BF16, tag="sin", name=f"sin{kc}_{g}")
                nc.sync.dma_start(t[:], d["s_in"].ap()[kc, :, b0:b0 + 2])
                s_sb.append(t)
        for bb in (b0, b0 + 1):
            for mc in range(KC):
                for y0, nr in ((0, 15), (15, 14)):
                    pss = ps_conv.tile([P, nr, 29], F32, tag="pss")
                    i = 0
                    for tap in range(9):
                        dy, dx = tap // 3, tap % 3
                        for kc in range(KC):
                            lhsT = csw_sb[kc][:, tap * 2 * P + mc * P:tap * 2 * P + (mc + 1) * P]
                            rhs = s_sb[kc][:, bb - b0, y0 + dy:y0 + dy + nr, dx:dx + 29]
                            nc.tensor.matmul(pss[:], lhsT, rhs,
                                             start=(i == 0), stop=(i == 17))
                            i += 1
                    nc.scalar.activation(srelu_sb[mc][:, bb, y0:y0 + nr, 0:29],
                                         pss[:], AF.Relu, bias=csb_sb[mc][:])

        if g == 0:
            # conv_kernel (3x3, BN+ReLU folded): krelu[mc] = [128, b*25+tap]
            for mc in range(KC):
                psk = ps_conv.tile([P, BPC, 5, 5], F32, tag="pss")
                i = 0
                for tap in range(9):
                    dy, dx = tap // 3, tap % 3
                    for kc in range(KC):
                        lhsT = ckw_sb[kc][:, tap * 2 * P + mc * P:tap * 2 * P + (mc + 1) * P]
                        rhs = k_sb[kc][:, :, dy:dy + 5, dx:dx + 5]
                        nc.tensor.matmul(psk[:], lhsT, rhs, start=(i == 0), stop=(i == 17))
                        i += 1
                nc.scalar.activation(krelu_sb[mc][:], psk[:], AF.Relu,
                                     bias=ckb_sb[mc][:])
                # doubled bf16 copy (k2[p, 2t] = k2[p, 2t+1] = krelu[p, t])
                # for the broadcast-AP diag builds
                k2out = _bcast_ap(k2_sb[mc][:], [[2, BPC * 25], [1, 2]])
                k2in = _bcast_ap(psk[:].rearrange("p b y x -> p (b y x)"),
                                 [[1, BPC * 25], [0, 2]])
                nc.scalar.activation(k2out, k2in, AF.Relu, bias=ckb_sb[mc][:])
            emit_diags(0)
            emit_diags(1)
        if g + 1 < NG:
            emit_diags(2 * g + 2)
            emit_diags(2 * g + 3)
        # heads BEFORE this iteration's xcorr chains: their ScalarE ops must
        # not queue behind ~40us of 'm'-pair copies or ps_hd backs up PE
        if g >= 2:
            emit_head(b0 - 4)
            emit_head(b0 - 3)
        if g >= 1:
            emit_xcorr(b0 - 2)
            emit_xcorr(b0 - 1)

    emit_head(BPC - 4)
    emit_head(BPC - 3)
    emit_xcorr(BPC - 2)
    emit_xcorr(BPC - 1)
    emit_head(BPC - 2)
    emit_head(BPC - 1)

    nc.sync.dma_start(d["out"].ap(), out_sb[:])


def _build_program():
    if "nc" in _prog_cache:
        return _prog_cache["nc"]
    nc = bacc.Bacc("TRN2", target_bir_lowering=False, debug=False,
                   num_devices=NCORES)
    d = {}
    d["s_in"] = nc.dram_tensor("s_in", [KC, P, BPC, 31, 32], BF16, kind="ExternalInput")
    d["k_in"] = nc.dram_tensor("k_in", [KC, P, BPC, 9, 9], BF16, kind="ExternalInput")
    d["csw"] = nc.dram_tensor("csw", [KC, P, 9, 2, P], BF16, kind="ExternalInput")
    d["ckw"] = nc.dram_tensor("ckw", [KC, P, 9, 2, P], BF16, kind="ExternalInput")
    d["cs_bias"] = nc.dram_tensor("cs_bias", [KC, P, 1], F32, kind="ExternalInput")
    d["ck_bias"] = nc.dram_tensor("ck_bias", [KC, P, 1], F32, kind="ExternalInput")
    d["h1w"] = nc.dram_tensor("h1w", [KC, P, 2, P], F16, kind="ExternalInput")
    d["h1_bias"] = nc.dram_tensor("h1_bias", [KC, P, 1], F32, kind="ExternalInput")
    d["h2w"] = nc.dram_tensor("h2w", [KC, P, OUT], F16, kind="ExternalInput")
    d["h2_bias"] = nc.dram_tensor("h2_bias", [OUT, 1], F32, kind="ExternalInput")
    d["ident"] = nc.dram_tensor("ident", [P, P], BF16, kind="ExternalInput")
    d["out"] = nc.dram_tensor("out", [OUT, BPC * 625], F32, kind="ExternalOutput")

    with tile.TileContext(nc) as tc:
        with ExitStack() as ctx:
            _emit(nc, tc, ctx, d)
    nc.compile()
    _prog_cache["nc"] = nc
    return nc


def kernel(**inputs):
    global LAST_RESULTS
    f32 = lambda x: np.ascontiguousarray(np.asarray(x), dtype=np.float32)
    kern, search = f32(inputs["kernel"]), f32(inputs["search"])

    # fold BN into conv weights / biases
    cks = f32(inputs["ck_g"]) / np.sqrt(f32(inputs["ck_v"]) + EPS)
    ckw_f = f32(inputs["ck_w"]) * cks[:, None, None, None]
    ckb = f32(inputs["ck_b"]) - f32(inputs["ck_m"]) * cks
    css = f32(inputs["cs_g"]) / np.sqrt(f32(inputs["cs_v"]) + EPS)
    csw_f = f32(inputs["cs_w"]) * css[:, None, None, None]
    csb = f32(inputs["cs_b"]) - f32(inputs["cs_m"]) * css
    h1s = f32(inputs["h_g"]) / np.sqrt(f32(inputs["h_v"]) + EPS)
    h1w_f = f32(inputs["h1_w"]) * h1s[:, None]
    h1b = f32(inputs["h_b"]) - f32(inputs["h_m"]) * h1s

    shared = {
        "csw": np.ascontiguousarray(
            csw_f.transpose(1, 2, 3, 0).reshape(KC, P, 9, 2, P)).astype(ml_dtypes.bfloat16),
        "ckw": np.ascontiguousarray(
            ckw_f.transpose(1, 2, 3, 0).reshape(KC, P, 9, 2, P)).astype(ml_dtypes.bfloat16),
        "cs_bias": csb.reshape(KC, P, 1),
        "ck_bias": ckb.reshape(KC, P, 1),
        "h1w": np.ascontiguousarray(
            h1w_f.transpose(1, 0).reshape(KC, P, 2, P)).astype(np.float16),
        "h1_bias": h1b.reshape(KC, P, 1),
        "h2w": np.ascontiguousarray(
            f32(inputs["h2_w"]).transpose(1, 0).reshape(KC, P, OUT)).astype(np.float16),
        "h2_bias": f32(inputs["h2_b"]).reshape(OUT, 1),
        "ident": np.eye(P, dtype=ml_dtypes.bfloat16),
    }
    in_maps = []
    for i in range(NCORES):
        sl = slice(i * BPC, (i + 1) * BPC)
        m = dict(shared)
        s_pad = np.zeros((KC, P, BPC, 31, 32), ml_dtypes.bfloat16)
        s_pad[..., :31] = search[sl].transpose(1, 0, 2, 3).reshape(KC, P, BPC, 31, 31)
        m["s_in"] = s_pad
        k_pad = np.zeros((KC, P, BPC, 9, 9), ml_dtypes.bfloat16)
        k_pad[..., :7, :7] = kern[sl].transpose(1, 0, 2, 3).reshape(KC, P, BPC, 7, 7)
        m["k_in"] = k_pad
        in_maps.append(m)

    nc = _build_program()
    res = run_bass_kernel_spmd(nc, in_maps, core_ids=list(range(NCORES)))
    LAST_RESULTS = res
    out = np.empty((B, OUT, 25, 25), dtype=np.float32)
    for i in range(NCORES):
        o = res.results[i]["out"].reshape(OUT, BPC, 25, 25)
        out[i * BPC:(i + 1) * BPC] = o.transpose(1, 0, 2, 3)
    return out
